# revision 26
# baseline (speedup 1.0000x reference)
"""ARGenerator TRN2 kernel.

Math (per batch row b):
  h1 = relu(x @ W1.T + b1); h2 = relu(h1 @ W2.T + b2)
  mlp = tanh(h2 @ W3.T + b3)
  ar[t] = noise[t] + sum_i c[i] * ar[t-1-i]  (zero-init, t >= 7; 0 for t < 7)
  out = mlp + ar

The AR recurrence is linear time-invariant -> ar = conv(noise_masked, h)
with h the (geometrically decaying) impulse response, truncated at
(nb-1)*128 taps.  The conv becomes nb banded 128x128 Toeplitz matmuls
per output time-tile, fully parallel over time.

Layout strategy (pure data parallel over 8 cores, B_shard = 1024):
  Everything runs in bf16 (tolerance is 2e-2; bf16 keeps us ~5e-3) and
  in TRANSPOSED activation layout [feature/time on partitions, batch on
  free dim].  The host pre-transposes x and noise shards, so the device
  does plain contiguous DMA loads and zero PE transposes.  The output
  is produced transposed (which lets the b3 bias + tanh fuse into one
  ACT instruction with a per-partition bias) and the host transposes it
  back.

Pipeline: the batch shard is processed in 2 chunks of 512 so that
chunk B's input DMA and layer-1 matmuls overlap chunk A's t-loop
(the t-loop is wire-paced, leaving PE idle slack that exactly fits
L1(B)); chunk A's stores likewise overlap chunk B's loads.  Per-core
wire traffic: 1MB W1 + 1.1MB consts + 8MB xT + 8MB nT + 8MB out.

Per-chunk phases:
  1. L1: psh1[h, b] += W1l_k.T @ xT_k over 32 k-tiles; ACT relu+b1.
  2. L2: one matmul + ACT relu+b2 -> h2T [128, 512] bf16.
  3. For each t-tile m (32):
       psc = sum_j Hb_j.T @ nT[m+j]     (banded conv, nb matmuls)
       psm = W3l[:, m].T @ h2T          (one matmul)
       th  = tanh(psm + b3[m])          (ACT, per-partition bias)
       out = th + psc  (DVE, the only engine that can read PSUM)
       -> bf16, stores merged x2 on the gpsimd queue.
"""

import numpy as np
import ml_dtypes

import concourse.bass as bass
import concourse.tile as tile
import concourse.mybir as mybir
from concourse import bacc

F32 = mybir.dt.float32
BF16 = mybir.dt.bfloat16
F8 = mybir.dt.float8e4
BF16_NP = ml_dtypes.bfloat16
F8_NP = ml_dtypes.float8_e4m3
W1_SCALE = 64.0


def impulse_response(c, s_out, tail_tol=1e-4):
    """Return (h, nb) with (nb-1)*128 taps covering the response."""
    AR = len(c)
    c = np.asarray(c, np.float64)
    h = np.zeros(s_out, np.float64)
    h[0] = 1.0
    for j in range(1, s_out):
        acc = 0.0
        for i in range(AR):
            if j - 1 - i >= 0:
                acc += c[i] * h[j - 1 - i]
        h[j] = acc
    L = 128
    while L < s_out and np.abs(h[L:]).sum() > tail_tol:
        L += 128
    # nb = number of 128-wide band blocks per output tile: the in-tile block
    # (j=0) plus one per preceding input tile the L-tap history reaches into.
    return h, L // 128 + 1


def band_blocks(h, nb):
    """Hb [128, nb*128]: block jj (for input-tile offset j = jj - (nb-1))
    has Hb[k_rel, t_rel] = h[t_rel - k_rel - 128*j] (0 <= lag < (nb-1)*128)."""
    L = (nb - 1) * 128
    a = np.arange(128)[:, None]   # k_rel
    b = np.arange(128)[None, :]   # t_rel
    blocks = []
    for jj in range(nb):
        j = jj - (nb - 1)
        lag = b - a - 128 * j
        m = (lag >= 0) & (lag < L)
        blk = np.where(m, np.take(np.pad(h[:L], (0, 1)), np.clip(lag, 0, L)), 0.0)
        blocks.append(blk)
    return np.concatenate(blocks, axis=1)


def host_prepare(W1, b1, W2, b2, W3, b3, ar_coef, S_IN, S_OUT, H):
    """Small device tensors in exactly the SBUF layout used, bf16."""
    n_s = S_IN // 128
    # W1l[p, k*H + h] = W1[h, k*128 + p]  (lhsT tiles for layer 1)
    W1l = np.ascontiguousarray(
        W1.reshape(H, n_s, 128).transpose(2, 1, 0).reshape(128, n_s * H)
    )
    h, nb = impulse_response(ar_coef, S_OUT)
    return {
        "W1l": (W1l * W1_SCALE).astype(F8_NP),  # fp8, scaled into e4m3 normal range
        "W2l": np.ascontiguousarray(W2.T).astype(BF16_NP),   # [H_in, H_out]
        "W3l": np.ascontiguousarray(W3.T).astype(BF16_NP),   # [H, S_OUT]
        "b1c": np.ascontiguousarray(b1.reshape(H, 1), dtype=np.float32),
        "b2c": np.ascontiguousarray(b2.reshape(H, 1), dtype=np.float32),
        "b3m": np.ascontiguousarray(b3.reshape(S_OUT // 128, 128).T,
                                    dtype=np.float32),       # [128, n_t]
        "Hb": band_blocks(h, nb).astype(BF16_NP),
    }, nb


def build_kernel(B_shard, S_IN, S_OUT, H, nb):
    P = 128
    NC = 2                        # batch chunks
    CW = B_shard // NC            # chunk width (free dim of every op)
    assert H == P and CW == 512
    n_s = S_IN // P               # 32 input k-tiles
    n_t = S_OUT // P              # 32 output t-tiles

    nc = bacc.Bacc(trn_type="TRN2", target_bir_lowering=False, debug=False)

    xT_d = nc.dram_tensor("xT", [NC, S_IN, CW], F8, kind="ExternalInput").ap()
    nT_d = nc.dram_tensor("nT", [NC, S_OUT, CW], BF16, kind="ExternalInput").ap()
    W1_d = nc.dram_tensor("W1l", [P, n_s * H], F8, kind="ExternalInput").ap()
    W2_d = nc.dram_tensor("W2l", [H, H], BF16, kind="ExternalInput").ap()
    W3_d = nc.dram_tensor("W3l", [H, S_OUT], BF16, kind="ExternalInput").ap()
    b1_d = nc.dram_tensor("b1c", [H, 1], F32, kind="ExternalInput").ap()
    b2_d = nc.dram_tensor("b2c", [H, 1], F32, kind="ExternalInput").ap()
    b3_d = nc.dram_tensor("b3m", [P, n_t], F32, kind="ExternalInput").ap()
    Hb_d = nc.dram_tensor("Hb", [P, nb * P], BF16, kind="ExternalInput").ap()
    out_d = nc.dram_tensor("outT", [NC, S_OUT, CW], BF16,
                           kind="ExternalOutput").ap()

    with tile.TileContext(nc) as tc:
        with tc.tile_pool(name="const", bufs=1) as cpool:
            # W1 split into 4 tiles across both queues: dependency tracking
            # is per-tile, so a monolithic W1 would stall layer-1 k=0 until
            # the whole 1MB lands.
            W1t = []
            for i in range(4):
                w = cpool.tile([P, (n_s // 4) * H], F8, tag=f"w1_{i}")
                (nc.sync if i % 2 == 0 else nc.scalar).dma_start(
                    w[:], W1_d[:, i * (n_s // 4) * H:(i + 1) * (n_s // 4) * H])
                W1t.append(w)

            def W1sl(k):
                return W1t[k // 8][:, (k % 8) * H:(k % 8 + 1) * H]
            # scalar queue: small consts, then it helps carry chunk-A x.
            W2s = cpool.tile([H, H], BF16, tag="w2")
            nc.scalar.dma_start(W2s[:], W2_d[:])
            b1s = cpool.tile([H, 1], F32, tag="b1")
            nc.scalar.dma_start(b1s[:], b1_d[:])
            b2s = cpool.tile([H, 1], F32, tag="b2")
            nc.scalar.dma_start(b2s[:], b2_d[:])
            # t-loop consts go on the gpsimd queue (idle until stores begin):
            # on scalar they would delay the chunk-A x tiles behind 1.2MB.
            b3s = cpool.tile([P, n_t], F32, tag="b3")
            nc.gpsimd.dma_start(b3s[:], b3_d[:])
            Hbs = cpool.tile([P, nb * P], BF16, tag="hb")
            nc.gpsimd.dma_start(Hbs[:], Hb_d[:])
            # W3 is only needed when the t-loop starts (~34us): issuing it
            # here would steal early wire bandwidth from the chunk-A x tiles.
            W3s = cpool.tile([H, S_OUT], BF16, tag="w3")

            with (
                tc.tile_pool(name="warm", bufs=1) as wpool,
                tc.tile_pool(name="xT", bufs=n_s // 4) as xTp,
                tc.tile_pool(name="nT", bufs=8) as nTp,
                tc.tile_pool(name="act", bufs=2) as actp,
                tc.tile_pool(name="th", bufs=8) as thp,
                tc.tile_pool(name="outT", bufs=8) as outp,
                tc.tile_pool(name="psA", bufs=4, space="PSUM") as psA,
                tc.tile_pool(name="psB", bufs=4, space="PSUM") as psB,
            ):
                # ---- PE warm-up: the HAM clock gate defaults the PE array
                # to 1.2 GHz and only releases 2.4 GHz after ~3.4us of
                # sustained matmul activity; it re-throttles after ~3.4us
                # idle.  The first real matmul cannot start until W1+x
                # arrive (~13us: framework init + DMA), so keep the PE busy
                # on a zeroed scratch tile until then -- otherwise the whole
                # layer-1 phase runs at half clock.
                wsrc = wpool.tile([P, 4 * P], BF16, tag="wsrc")
                nc.vector.memset(wsrc[:], 0.0)
                wsnk = wpool.tile([P, 4], F32, tag="wsnk")
                psw = psB.tile([P, CW], F32, tag="ps", name="psw")
                for i in range(20):
                    nc.tensor.matmul(psw[:], wsrc[:, :P], wsrc[:])

                # ---- chunked input loads: 4 k-tiles per DMA (512KB)
                xts = {}

                def load_x4(c, g, eng):
                    t = xTp.tile([P, 4, CW], F8, tag="xt", name=f"xt{c}_{g}")
                    src = xT_d[c, g * 4 * P:(g + 1) * 4 * P, :].rearrange(
                        "(blk p) f -> p blk f", p=P)
                    eng.dma_start(t[:], src)
                    xts[c, g] = t

                def xt(c, k):
                    return xts[c, k // 4][:, k % 4, :]

                ntm = {}

                def load_n4(c, g):
                    t = nTp.tile([P, 4, CW], BF16, tag="nt", name=f"nt{c}_{g}")
                    src = nT_d[c, g * 4 * P:(g + 1) * 4 * P, :].rearrange(
                        "(blk p) f -> p blk f", p=P)
                    nc.sync.dma_start(t[:], src)
                    ntm[c, g] = t

                def nt(c, m):
                    return ntm[c, m // 4][:, m % 4, :]

                # chunk A x: alternate sync/scalar so one queue's trigger
                # rate doesn't cap the wire.
                for g in range(n_s // 4):
                    load_x4(0, g, nc.sync if g % 2 == 0 else nc.scalar)
                nc.gpsimd.dma_start(W3s[:], W3_d[:])
                # chunk A noise + chunk B x, interleaved on sync: both are
                # consumed at ~1 tile per t-loop-A step.
                load_n4(0, 0)
                for g in range(n_s // 4):
                    load_x4(1, g, nc.sync)
                    if g + 1 < 8:
                        load_n4(0, g + 1)

                def layer12(c, psh1):
                    h1T = actp.tile([H, CW], BF16, tag="act", name=f"h1T{c}")
                    # scale undoes the x64 put on W1 to lift fp8 denormals
                    nc.scalar.activation(
                        h1T[:], psh1[:], mybir.ActivationFunctionType.Relu,
                        bias=b1s[:], scale=1.0 / W1_SCALE,
                    )
                    psh2 = psA.tile([H, CW], F32, tag="psA", name=f"psh2{c}")
                    nc.tensor.matmul(psh2[:], W2s[:], h1T[:])
                    h2T = actp.tile([H, CW], BF16, tag="act", name=f"h2T{c}")
                    nc.scalar.activation(
                        h2T[:], psh2[:], mybir.ActivationFunctionType.Relu,
                        bias=b2s[:],
                    )
                    return h2T

                # ---- L1(A) as one dense run; L1(B) is interleaved into
                # t-loop-A below (1 matmul per t-tile) to use the PE slack
                # while that loop is wire-paced.
                psh1a = psA.tile([H, CW], F32, tag="psA", name="psh1a")
                for k in range(n_s):
                    nc.tensor.matmul(
                        psh1a[:], W1sl(k), xt(0, k),
                        start=(k == 0), stop=(k == n_s - 1),
                    )
                    if k % 5 == 4:
                        # filler matmul on already-present data: keeps the
                        # HAM clock warm while L1(A) waits on the DMA ramp;
                        # absorbed by wire pacing, never on the critical path
                        nc.tensor.matmul(
                            psw[:], W1t[k // 8][:, :P], W1t[k // 8][:, :CW])
                nc.vector.tensor_copy(wsnk[:], psw[:, :4])
                h2Ta = layer12(0, psh1a)
                psh1b = psA.tile([H, CW], F32, tag="psA", name="psh1b")

                def t_loop(c, h2T, extra=None):
                    for m in range(n_t):
                        if m % 4 == 0:
                            g = (m + 8) // 4
                            if c == 0:
                                # chunk-A groups were all queued up front
                                # (pool backpressure self-paces them)
                                if g >= n_t // 4:
                                    load_n4(1, g - n_t // 4)
                            elif g < n_t // 4:
                                load_n4(c, g)
                        jlist = [j for j in range(-(nb - 1), 1) if m + j >= 0]
                        psc = psB.tile([P, CW], F32, tag="ps",
                                       name=f"psc{c}_{m}")
                        for i, j in enumerate(jlist):
                            jj = j + nb - 1
                            nc.tensor.matmul(
                                psc[:], Hbs[:, jj * P:(jj + 1) * P],
                                nt(c, m + j),
                                start=(i == 0), stop=(i == len(jlist) - 1),
                            )
                        psm = psA.tile([P, CW], F32, tag="psA",
                                       name=f"psm{c}_{m}")
                        nc.tensor.matmul(
                            psm[:], W3s[:, m * P:(m + 1) * P], h2T[:],
                        )
                        if extra is not None:
                            extra(m)
                        th = thp.tile([P, CW], BF16, tag="th")
                        nc.scalar.activation(
                            th[:], psm[:], mybir.ActivationFunctionType.Tanh,
                            bias=b3s[:, m:m + 1],
                        )
                        if m % 2 == 0:
                            ot = outp.tile([P, 2, CW], BF16, tag="ot",
                                           name=f"ot{c}_{m // 2}")
                        nc.vector.tensor_add(ot[:, m % 2, :], th[:], psc[:])
                        if m % 2 == 1:
                            dst = out_d[c, (m - 1) * P:(m + 1) * P, :].rearrange(
                                "(blk p) f -> p blk f", p=P)
                            nc.gpsimd.dma_start(dst, ot[:])

                def l1b_step(m):
                    nc.tensor.matmul(
                        psh1b[:], W1sl(m), xt(1, m),
                        start=(m == 0), stop=(m == n_s - 1),
                    )

                t_loop(0, h2Ta, extra=l1b_step)
                h2Tb = layer12(1, psh1b)
                t_loop(1, h2Tb)

    nc.compile()
    return nc


# ---------------------------------------------------------------------------
# Self-contained kernel() entry point (the graded contract).
# ---------------------------------------------------------------------------

N_CORES = 8
_B, _S_IN, _S_OUT, _H, _AR = 8192, 4096, 4096, 128, 7
_CW = 512

_CACHE = {}


def _prep_and_build(inputs):
    dev, nb = host_prepare(
        np.asarray(inputs["W1"], np.float32), np.asarray(inputs["b1"], np.float32),
        np.asarray(inputs["W2"], np.float32), np.asarray(inputs["b2"], np.float32),
        np.asarray(inputs["W3"], np.float32), np.asarray(inputs["b3"], np.float32),
        np.asarray(inputs["ar_coef"], np.float32),
        _S_IN, _S_OUT, _H,
    )
    B_total = inputs["x"].shape[0]
    B_shard = B_total // N_CORES
    key = (B_shard, nb)
    if key not in _CACHE:
        _CACHE[key] = build_kernel(B_shard, _S_IN, _S_OUT, _H, nb)
    return _CACHE[key], dev, B_shard


def _chunked_T(a, B_shard, dt=BF16_NP):
    """[B_shard, S] fp32 -> [2, S, B_shard//2] transposed chunks."""
    aT = a.astype(dt).T                           # [S, B_shard]
    cw = B_shard // 2
    return np.ascontiguousarray(
        np.stack([aT[:, :cw], aT[:, cw:]]))


def _in_maps(inputs, dev, B_shard):
    x = np.asarray(inputs["x"], np.float32)
    noise_m = np.asarray(inputs["noise"], np.float32).copy()
    noise_m[:, :_AR] = 0.0
    maps = []
    for c in range(N_CORES):
        sl = slice(c * B_shard, (c + 1) * B_shard)
        m = {"xT": _chunked_T(x[sl], B_shard, F8_NP),
             "nT": _chunked_T(noise_m[sl], B_shard)}
        m.update(dev)
        maps.append(m)
    return maps


def kernel(**inputs):
    nc, dev, B_shard = _prep_and_build(inputs)
    maps = _in_maps(inputs, dev, B_shard)
    import concourse.bass_utils as bass_utils

    res = bass_utils.run_bass_kernel_spmd(
        nc, maps, core_ids=list(range(N_CORES)), trace=False
    )
    shards = []
    for c in range(N_CORES):
        o = np.asarray(res.results[c]["outT"])    # [2, S_OUT, CW] bf16
        shards.append(np.concatenate([o[0].T, o[1].T], axis=0))
    return np.concatenate(shards, axis=0).astype(np.float32)


def run_traced(inputs):
    """Profiled run (NTFF -> neuron-profile) for the local test harness."""
    import contextlib
    import ctypes
    import sys as _sys
    import types as _types

    so = "/opt/axon/libaxon_pjrt.so"
    if "antenv.axon_hooks" not in _sys.modules:
        try:
            lib2 = ctypes.CDLL(so)
            lib2.axon_start_nrt_profile.argtypes = [
                ctypes.POINTER(ctypes.c_int64), ctypes.c_size_t]
            lib2.axon_start_nrt_profile.restype = ctypes.c_int64
            lib2.axon_stop_nrt_profile.argtypes = [ctypes.c_char_p]
            lib2.axon_stop_nrt_profile.restype = ctypes.c_int64

            @contextlib.contextmanager
            def _hook(output_dir, device_ids):
                import jax
                jax.devices()
                if device_ids:
                    ids_arr = (ctypes.c_int64 * len(device_ids))(*device_ids)
                    rc = lib2.axon_start_nrt_profile(ids_arr, len(device_ids))
                else:
                    rc = lib2.axon_start_nrt_profile(None, 0)
                if rc != 0:
                    raise RuntimeError(f"axon_start_nrt_profile rc={rc}")
                try:
                    yield
                finally:
                    lib2.axon_stop_nrt_profile(str(output_dir).encode())

            mod = _types.ModuleType("antenv.axon_hooks")
            mod.get_axon_ntff_profile_hook = lambda: _hook
            mod.set_axon_ntff_profile_hook = lambda h: None
            _sys.modules["antenv.axon_hooks"] = mod
        except OSError:
            pass
    import concourse.bass_utils as bass_utils
    bass_utils.upload_artifacts = lambda tmpdir: tmpdir

    nc, dev, B_shard = _prep_and_build(inputs)
    maps = _in_maps(inputs, dev, B_shard)
    return bass_utils.run_bass_kernel_spmd(
        nc, maps, core_ids=list(range(N_CORES)), trace=True, trace_cores=[0]
    )
